# revision 13
# baseline (speedup 1.0000x reference)
"""CPM3 attention kernel for 8 trn2 NeuronCores.

Sharding: tensor-parallel over heads (2 heads/core x both batches).
Device computes per-core partial outputs (Wo row-sharded); host sums.

Structure (v2):
- mask+bias are folded on the host into one multiplicative fp16 table
  eb = mask * exp(position_bias), using exp(s + pb) * mask == exp(s) * eb.
  This removes the per-tile DVE mask/bias fuse and the PSUM identity-matmul
  injection of the additive design (big PE + DVE savings).
- scores are computed transposed [k, q] so the softmax needs no
  partition-dim reductions; denominators come free from an all-ones block
  appended to the transposed-V stationary (one extra PSUM partition).
- per (b, head): the k-sweep runs in 8 chunks of 2 k-tiles: 2 QK matmuls
  into one 2-bank PSUM tile, a single EXP (Act engine) over [128,1024],
  a single fp16 multiply by eb (DVE 2x_1P mode), then 2 PV matmuls.
- hv is transposed k-major via the DMA xbar (SBUF->SBUF), not the PE.
- fp16 operands for all matmuls (2-byte weights use the PE background
  weight-load path and halve HBM traffic); PSUM accumulation stays fp32.
"""

import sys

sys.path.insert(0, "/opt/trn_rl_repo")

import numpy as np

import concourse.bass as bass
import concourse.bacc as bacc
import concourse.tile as tile
import concourse.mybir as mybir
from concourse.bass_utils import run_bass_kernel_spmd

B, L, D, H, DH = 2, 2048, 1024, 16, 64
N_CORES = 8
HPC = H // N_CORES  # heads per core = 2
QTS = 512  # q tile size
QN = L // QTS  # 4
KN = L // 128  # 16 k-tiles
KPN = KN // 2  # 8 k-pairs (2 k-tiles share one 2-bank PSUM score tile)
DC = D // 128  # 8 contraction chunks
HVW = 256  # transposed-V columns per k-tile: [v_h0 | ones64 | v_h1 | ones64]

F32 = mybir.dt.float32
F32R = mybir.dt.float32r
F16 = mybir.dt.float16

_CACHE: dict = {}


def _build():
    if "nc" in _CACHE:
        return _CACHE["nc"]
    nc = bacc.Bacc("TRN2", target_bir_lowering=False, debug=False, num_devices=N_CORES)

    qT = nc.dram_tensor("qT", [B, DC, 128, L], F16, kind="ExternalInput").ap()
    kvT = nc.dram_tensor("kvT", [B, DC, 128, L], F16, kind="ExternalInput").ap()
    wq = nc.dram_tensor("wq", [128, DC, 128], F16, kind="ExternalInput").ap()
    wk = nc.dram_tensor("wk", [128, DC, 128], F16, kind="ExternalInput").ap()
    wv = nc.dram_tensor("wv", [128, DC, 128], F16, kind="ExternalInput").ap()
    wo = nc.dram_tensor("wo", [128, D], F16, kind="ExternalInput").ap()
    eb = nc.dram_tensor(
        "eb", [QN, B, HPC, 128, KPN, 1024], F16, kind="ExternalInput"
    ).ap()
    indh = nc.dram_tensor("indh", [1, 128], F16, kind="ExternalInput").ap()
    identr = nc.dram_tensor("identr", [128, 128], F32R, kind="ExternalInput").ap()
    out = nc.dram_tensor("out", [B, L, D], F16, kind="ExternalOutput").ap()

    EXP = mybir.ActivationFunctionType.Exp
    MULT = mybir.AluOpType.mult

    with tile.TileContext(nc) as tc:
        with (
            tc.tile_pool(name="const", bufs=1) as constp,
            tc.tile_pool(name="stage", bufs=2) as stagep,
            tc.tile_pool(name="qb1", bufs=1) as qb1p,
            tc.tile_pool(name="kvb1", bufs=1) as kvb1p,
            tc.tile_pool(name="hq", bufs=2) as hqp,
            tc.tile_pool(name="hk", bufs=2) as hkp,
            tc.tile_pool(name="hvl", bufs=1) as hvlp,
            tc.tile_pool(name="hvt", bufs=2) as hvtp,
            tc.tile_pool(name="ebp", bufs=2) as ebp,
            tc.tile_pool(name="p0", bufs=3) as p0p,
            tc.tile_pool(name="pt", bufs=4) as ptp,
            tc.tile_pool(name="ctxn", bufs=2) as ctxnp,
            tc.tile_pool(name="rc", bufs=4) as rcp,
            tc.tile_pool(name="outb", bufs=2) as outp,
            tc.tile_pool(name="psA", bufs=2, space=bass.MemorySpace.PSUM) as psA,
            tc.tile_pool(name="psB", bufs=4, space=bass.MemorySpace.PSUM) as psB,
        ):
            # ---- eb prefetch for the first segments (segments are b-major) ----
            segs = [
                (b, qt, h) for b in range(B) for qt in range(QN) for h in range(HPC)
            ]
            eb_t = {}

            def fetch_eb(i):
                if i >= len(segs):
                    return
                b, qt, h = segs[i]
                t = ebp.tile([128, KPN, 1024], F16, tag="eb", name=f"eb{b}_{qt}_{h}")
                nc.gpsimd.dma_start(t[:], eb[qt, b, h])
                eb_t[(b, qt, h)] = t

            # ---- constants (wq first: the first projection needs only it) ----
            wq_t = constp.tile([128, DC, 128], F16, tag="wq")
            nc.sync.dma_start(wq_t[:], wq[:])
            indh_t = constp.tile([1, 128], F16, tag="indh")
            nc.sync.dma_start(indh_t[:], indh[:])
            wk_t = constp.tile([128, DC, 128], F16, tag="wk")
            nc.sync.dma_start(wk_t[:], wk[:])
            wv_t = constp.tile([128, DC, 128], F16, tag="wv")
            nc.sync.dma_start(wv_t[:], wv[:])
            wo_t = constp.tile([128, D], F16, tag="wo")
            nc.sync.dma_start(wo_t[:], wo[:])
            identr_t = constp.tile([128, 128], F32R, tag="identr")
            nc.sync.dma_start(identr_t[:], identr[:])

            def emit_pv(b, h, ctx, kp, pt):
                for ki in range(2):
                    kt = 2 * kp + ki
                    nc.tensor.matmul(
                        ctx[:],
                        hvT[b][:, kt, 2 * h : 2 * h + 2, :],
                        pt[:, ki * QTS : (ki + 1) * QTS],
                        start=(kt == 0),
                        stop=(kt == KN - 1),
                    )

            # ---- prologue helpers (used inline for b0, as interleaved chunks for b1)
            hq_sb, hk_sb, hvT, hvf = {}, {}, {}, {}

            def emit_hq(b, get_c):
                hq_ps = [
                    psA.tile([128, 1024], F32, tag="A", name=f"hqps{b}_{i}")
                    for i in range(2)
                ]
                for dc in range(DC):
                    c = get_c(dc)
                    for qt in range(QN):
                        nc.tensor.matmul(
                            hq_ps[qt // 2][:, (qt % 2) * QTS : (qt % 2 + 1) * QTS],
                            wq_t[:, dc, :],
                            c[:, qt * QTS : (qt + 1) * QTS],
                            start=(dc == 0),
                            stop=(dc == DC - 1),
                        )
                hq_sb[b] = hqp.tile([128, L], F16, tag="hq", name=f"hq{b}")
                for i in range(2):
                    nc.vector.tensor_copy(
                        hq_sb[b][:, i * 1024 : (i + 1) * 1024], hq_ps[i][:]
                    )

            def emit_hk(b, get_c, also_hv):
                hk_ps = [
                    psA.tile([128, 1024], F32, tag="A", name=f"hkps{b}_{i}")
                    for i in range(2)
                ]
                if also_hv:
                    hv_ps = [
                        psB.tile([128, QTS], F32, tag="B", name=f"hvps{b}_{i}")
                        for i in range(QN)
                    ]
                for dc in range(DC):
                    c = get_c(dc)
                    for qt in range(QN):
                        nc.tensor.matmul(
                            hk_ps[qt // 2][:, (qt % 2) * QTS : (qt % 2 + 1) * QTS],
                            wk_t[:, dc, :],
                            c[:, qt * QTS : (qt + 1) * QTS],
                            start=(dc == 0),
                            stop=(dc == DC - 1),
                        )
                        if also_hv:
                            nc.tensor.matmul(
                                hv_ps[qt][:],
                                wv_t[:, dc, :],
                                c[:, qt * QTS : (qt + 1) * QTS],
                                start=(dc == 0),
                                stop=(dc == DC - 1),
                            )
                hk_sb[b] = hkp.tile([128, L], F16, tag="hk", name=f"hk{b}")
                for i in range(2):
                    nc.vector.tensor_copy(
                        hk_sb[b][:, i * 1024 : (i + 1) * 1024], hk_ps[i][:]
                    )
                if also_hv:
                    emit_hv_drain(b, hv_ps)

            def emit_hv(b, get_c):
                hv_ps = [
                    psB.tile([128, QTS], F32, tag="B", name=f"hvps{b}_{i}")
                    for i in range(QN)
                ]
                for dc in range(DC):
                    c = get_c(dc)
                    for qt in range(QN):
                        nc.tensor.matmul(
                            hv_ps[qt][:],
                            wv_t[:, dc, :],
                            c[:, qt * QTS : (qt + 1) * QTS],
                            start=(dc == 0),
                            stop=(dc == DC - 1),
                        )
                emit_hv_drain(b, hv_ps)

            def emit_hv_drain(b, hv_ps):
                hvf[b] = hvlp.tile([128, L], F32R, tag="hvl", name=f"hvf{b}")
                for i in range(QN):
                    nc.vector.tensor_copy(
                        hvf[b][:, i * QTS : (i + 1) * QTS], hv_ps[i][:]
                    )

            def emit_hv_t(b):
                # k-major transposed V, per k-tile [v_h0 | ones64 | v_h1 | ones64]
                # so each head's 128-wide stationary slice carries its values
                # plus denominator (all-ones) columns
                hvT[b] = hvtp.tile([128, KN, 4, 64], F16, tag="hvt", name=f"hvt{b}")
                nc.gpsimd.memset(hvT[b][:].bitcast(mybir.dt.uint16), 0x3C00)
                for kt in range(KN):
                    tp = psB.tile([128, 2, 64], F32R, tag="B", name=f"tp{b}_{kt}")
                    nc.tensor.transpose(
                        tp[:], hvf[b][:, kt * 128 : (kt + 1) * 128], identr_t[:]
                    )
                    nc.vector.tensor_copy(hvT[b][:, kt, 0::2, :], tp[:])

            # ---- b0 prologue: stream q/kv chunks, hk+hv share one kv stream ----
            def stream_q0(dc):
                c = stagep.tile([128, L], F16, tag="stage")
                nc.sync.dma_start(c[:], qT[0, dc])
                return c

            def stream_kv0(dc):
                c = stagep.tile([128, L], F16, tag="stage")
                nc.sync.dma_start(c[:], kvT[0, dc])
                return c

            emit_hq(0, stream_q0)
            emit_hk(0, stream_kv0, also_hv=True)
            emit_hv_t(0)

            # ---- b1 inputs staged into persistent tiles (DMAs overlap b0 work)
            qb1 = qb1p.tile([128, DC, L], F16, tag="qb1")
            kvb1 = kvb1p.tile([128, DC, L], F16, tag="kvb1")
            for dc in range(DC):
                nc.sync.dma_start(qb1[:, dc, :], qT[1, dc])
                nc.sync.dma_start(kvb1[:, dc, :], kvT[1, dc])

            fetch_eb(0)

            # b1 prologue chunks, emitted between the first b0 segments so the
            # score pipeline keeps the Act/Vector engines fed while the PE
            # works through them
            chunks = [
                lambda: emit_hk(1, lambda dc: kvb1[:, dc, :], also_hv=False),
                lambda: emit_hv(1, lambda dc: kvb1[:, dc, :]),
                lambda: emit_hq(1, lambda dc: qb1[:, dc, :]),
                lambda: emit_hv_t(1),
            ]

            # ---- main loop: per (b, q-tile, head) one full k-sweep; PV lags
            # one k-pair behind QK/exp/mult so the in-order PE queue never
            # stalls on the Act/DVE stages
            ctxn_cur = None
            for si, (b, qt, h) in enumerate(segs):
                fetch_eb(si + 1)
                ebt = eb_t.pop((b, qt, h))
                ctx = psB.tile([128, QTS], F32, tag="B", name=f"ctx{b}_{qt}_{h}")
                pending = None
                for kp in range(KPN):
                    sc = psA.tile(
                        [128, 1024], F32, tag="A", name=f"sc{b}_{qt}_{h}_{kp}"
                    )
                    for ki in range(2):
                        kt = 2 * kp + ki
                        nc.tensor.matmul(
                            sc[:, ki * QTS : (ki + 1) * QTS],
                            hk_sb[b][h * DH : (h + 1) * DH, kt * 128 : (kt + 1) * 128],
                            hq_sb[b][h * DH : (h + 1) * DH, qt * QTS : (qt + 1) * QTS],
                            start=True,
                            stop=True,
                        )
                    p0 = p0p.tile([128, 1024], F16, tag="p0")
                    nc.scalar.activation(p0[:], sc[:], EXP)
                    pt = ptp.tile([128, 1024], F16, tag="pt")
                    nc.vector.tensor_tensor(pt[:], p0[:], ebt[:, kp, :], MULT)
                    if pending is not None:
                        emit_pv(b, h, ctx, *pending)
                    pending = (kp, pt)
                emit_pv(b, h, ctx, *pending)
                # epilogue for (b, qt, h): normalize into the h-half of ctxn
                if h == 0:
                    ctxn_cur = ctxnp.tile(
                        [128, QTS], F16, tag="ctxn", name=f"ctxn{b}_{qt}"
                    )
                dsb = rcp.tile([1, QTS], F32, tag="dsb")
                nc.vector.tensor_copy(dsb[:], ctx[DH : DH + 1, :])
                rcf = rcp.tile([1, QTS], F32, tag="rcf")
                nc.vector.reciprocal_approx_fast(rcf[:], dsb[:])
                rcr = rcp.tile([1, QTS], F16, tag="rcr")
                nc.vector.tensor_copy(rcr[:], rcf[:])
                bcw = psB.tile([128, QTS], F32, tag="B", name=f"bcw{b}_{qt}_{h}")
                nc.tensor.matmul(bcw[:], indh_t[:], rcr[:], start=True, stop=True)
                bc_sb = rcp.tile([64, QTS], F16, tag="bcsb")
                nc.vector.tensor_copy(bc_sb[:], bcw[0:DH, :])
                nc.vector.tensor_tensor(
                    ctxn_cur[h * DH : (h + 1) * DH, :], ctx[0:DH, :], bc_sb[:], MULT
                )
                if h == 1:
                    # output projection for (b, qt)
                    for qs in range(QN):
                        ob = outp.tile(
                            [128, D], F16, tag="outb", name=f"ob{b}_{qt}_{qs}"
                        )
                        for oh in range(2):
                            op = psB.tile(
                                [128, QTS], F32, tag="B", name=f"op{b}_{qt}_{qs}_{oh}"
                            )
                            nc.tensor.matmul(
                                op[:],
                                ctxn_cur[:, qs * 128 : (qs + 1) * 128],
                                wo_t[:, oh * QTS : (oh + 1) * QTS],
                                start=True,
                                stop=True,
                            )
                            if oh == 0:
                                nc.vector.tensor_copy(
                                    ob[:, oh * QTS : (oh + 1) * QTS], op[:]
                                )
                            else:
                                nc.scalar.copy(
                                    ob[:, oh * QTS : (oh + 1) * QTS], op[:]
                                )
                        r0 = qt * QTS + qs * 128
                        nc.sync.dma_start(out[b, r0 : r0 + 128, :], ob[:])
                if si < len(chunks):
                    chunks[si]()

    nc.compile()
    _CACHE["nc"] = nc
    return nc


def _prep_core(core, Wq, Wk, Wv, Wo, shared):
    """Per-core input map. `shared` holds core-independent packed arrays."""
    h0 = core * HPC
    rows = slice(h0 * DH, (h0 + HPC) * DH)

    def packw(w, scale=1.0):
        return np.ascontiguousarray(
            (w[rows].T * scale).reshape(DC, 128, 128).transpose(1, 0, 2)
        ).astype(np.float16)

    # eb[qt, b, hl, p, kp, ki*512+qf] = mask[b, q, k] * exp(pb[h, q, k])
    # with q = qt*512+qf, k = (2*kp+ki)*128+p  (fp16 bit arithmetic in uint16)
    expT, maskT = shared["expT"], shared["maskT"]
    ebc = np.empty((QN, B, HPC, 128, KPN, 1024), np.uint16)
    for qt in range(QN):
        for b in range(B):
            for hl in range(HPC):
                np.multiply(expT[h0 + hl, qt], maskT[b, qt], out=ebc[qt, b, hl])
    return {
        "qT": shared["qT"],
        "kvT": shared["kvT"],
        "indh": shared["indh"],
        "identr": shared["identr"],
        "wq": packw(Wq, 1.0 / np.sqrt(DH)),
        "wk": packw(Wk),
        "wv": packw(Wv),
        "wo": np.ascontiguousarray(Wo[:, rows].T).astype(np.float16),
        "eb": ebc.view(np.float16),
    }


def _prep_shared(query, key_value, mask, position_bias):
    qTp = np.ascontiguousarray(
        query.reshape(B, L, DC, 128).transpose(0, 2, 3, 1)
    ).astype(np.float16)
    kvTp = np.ascontiguousarray(
        key_value.reshape(B, L, DC, 128).transpose(0, 2, 3, 1)
    ).astype(np.float16)
    # [h, q, k] -> [h, qt, p, kp, ki, qf] (fp16 bits as uint16)
    expT = (
        np.ascontiguousarray(
            np.exp(position_bias.astype(np.float32))
            .astype(np.float16)
            .reshape(H, QN, QTS, KPN, 2, 128)
            .transpose(0, 1, 5, 3, 4, 2)
        )
        .view(np.uint16)
        .reshape(H, QN, 128, KPN, 1024)
    )
    maskT = np.ascontiguousarray(
        (np.asarray(mask, dtype=bool))
        .astype(np.uint16)
        .reshape(B, QN, QTS, KPN, 2, 128)
        .transpose(0, 1, 5, 3, 4, 2)
    ).reshape(B, QN, 128, KPN, 1024)
    indh = np.where(np.arange(128) < 64, 1.0, 0.0).astype(np.float16)[None, :]
    return {
        "qT": qTp,
        "kvT": kvTp,
        "expT": expT,
        "maskT": maskT,
        "indh": np.ascontiguousarray(indh),
        "identr": np.eye(128, dtype=np.float32),
    }


def kernel(query, key_value, mask, position_bias, Wq, Wk, Wv, Wo, _trace=False):
    query = np.asarray(query, dtype=np.float32)
    key_value = np.asarray(key_value, dtype=np.float32)
    mask = np.asarray(mask)
    position_bias = np.asarray(position_bias, dtype=np.float32)
    Wq = np.asarray(Wq, dtype=np.float32)
    Wk = np.asarray(Wk, dtype=np.float32)
    Wv = np.asarray(Wv, dtype=np.float32)
    Wo = np.asarray(Wo, dtype=np.float32)

    nc = _build()
    shared = _prep_shared(query, key_value, mask, position_bias)
    in_maps = [_prep_core(c, Wq, Wk, Wv, Wo, shared) for c in range(N_CORES)]
    res = run_bass_kernel_spmd(nc, in_maps, list(range(N_CORES)), trace=_trace)
    _CACHE["last_result"] = res
    acc = res.results[0]["out"].astype(np.float64)
    for c in range(1, N_CORES):
        acc += res.results[c]["out"]
    return acc.astype(np.float32)


# revision 14
# speedup vs baseline: 1.0106x; 1.0106x over previous
"""CPM3 attention kernel for 8 trn2 NeuronCores.

Sharding: tensor-parallel over heads (2 heads/core x both batches).
Device computes per-core partial outputs (Wo row-sharded); host sums.

Structure (v2):
- mask+bias are folded on the host into one multiplicative fp16 table
  eb = mask * exp(position_bias), using exp(s + pb) * mask == exp(s) * eb.
  This removes the per-tile DVE mask/bias fuse and the PSUM identity-matmul
  injection of the additive design (big PE + DVE savings).
- scores are computed transposed [k, q] so the softmax needs no
  partition-dim reductions; denominators come free from an all-ones block
  appended to the transposed-V stationary (one extra PSUM partition).
- per (b, head): the k-sweep runs in 8 chunks of 2 k-tiles: 2 QK matmuls
  into one 2-bank PSUM tile, a single EXP (Act engine) over [128,1024],
  a single fp16 multiply by eb (DVE 2x_1P mode), then 2 PV matmuls.
- hv is transposed k-major via the DMA xbar (SBUF->SBUF), not the PE.
- fp16 operands for all matmuls (2-byte weights use the PE background
  weight-load path and halve HBM traffic); PSUM accumulation stays fp32.
"""

import sys

sys.path.insert(0, "/opt/trn_rl_repo")

import numpy as np

import concourse.bass as bass
import concourse.bacc as bacc
import concourse.tile as tile
import concourse.mybir as mybir
from concourse.bass_utils import run_bass_kernel_spmd

B, L, D, H, DH = 2, 2048, 1024, 16, 64
N_CORES = 8
HPC = H // N_CORES  # heads per core = 2
QTS = 512  # q tile size
QN = L // QTS  # 4
KN = L // 128  # 16 k-tiles
KPN = KN // 2  # 8 k-pairs (2 k-tiles share one 2-bank PSUM score tile)
DC = D // 128  # 8 contraction chunks
HVW = 256  # transposed-V columns per k-tile: [v_h0 | ones64 | v_h1 | ones64]

F32 = mybir.dt.float32
F32R = mybir.dt.float32r
F16 = mybir.dt.float16

_CACHE: dict = {}


def _build():
    if "nc" in _CACHE:
        return _CACHE["nc"]
    nc = bacc.Bacc("TRN2", target_bir_lowering=False, debug=False, num_devices=N_CORES)

    qT = nc.dram_tensor("qT", [B, DC, 128, L], F16, kind="ExternalInput").ap()
    kvT = nc.dram_tensor("kvT", [B, DC, 128, L], F16, kind="ExternalInput").ap()
    wq = nc.dram_tensor("wq", [128, DC, 128], F16, kind="ExternalInput").ap()
    wk = nc.dram_tensor("wk", [128, DC, 128], F16, kind="ExternalInput").ap()
    wv = nc.dram_tensor("wv", [128, DC, 128], F16, kind="ExternalInput").ap()
    wo = nc.dram_tensor("wo", [128, D], F16, kind="ExternalInput").ap()
    eb = nc.dram_tensor(
        "eb", [QN, B, HPC, 128, KPN, 1024], F16, kind="ExternalInput"
    ).ap()
    indh = nc.dram_tensor("indh", [1, 128], F16, kind="ExternalInput").ap()
    identr = nc.dram_tensor("identr", [128, 128], F32R, kind="ExternalInput").ap()
    out = nc.dram_tensor("out", [B, L, D], F16, kind="ExternalOutput").ap()

    EXP = mybir.ActivationFunctionType.Exp
    MULT = mybir.AluOpType.mult

    with tile.TileContext(nc) as tc:
        with (
            tc.tile_pool(name="const", bufs=1) as constp,
            tc.tile_pool(name="stage", bufs=2) as stagep,
            tc.tile_pool(name="qb1", bufs=1) as qb1p,
            tc.tile_pool(name="kvb1", bufs=1) as kvb1p,
            tc.tile_pool(name="hq", bufs=2) as hqp,
            tc.tile_pool(name="hk", bufs=2) as hkp,
            tc.tile_pool(name="hvl", bufs=1) as hvlp,
            tc.tile_pool(name="hvt", bufs=2) as hvtp,
            tc.tile_pool(name="ebp", bufs=2) as ebp,
            tc.tile_pool(name="p0", bufs=3) as p0p,
            tc.tile_pool(name="pt", bufs=4) as ptp,
            tc.tile_pool(name="ctxn", bufs=2) as ctxnp,
            tc.tile_pool(name="rc", bufs=4) as rcp,
            tc.tile_pool(name="outb", bufs=2) as outp,
            tc.tile_pool(name="psA", bufs=2, space=bass.MemorySpace.PSUM) as psA,
            tc.tile_pool(name="psB", bufs=4, space=bass.MemorySpace.PSUM) as psB,
        ):
            # ---- eb prefetch for the first segments (segments are b-major) ----
            segs = [
                (b, qt, h) for b in range(B) for qt in range(QN) for h in range(HPC)
            ]
            eb_t = {}

            def fetch_eb(i):
                if i >= len(segs):
                    return
                b, qt, h = segs[i]
                t = ebp.tile([128, KPN, 1024], F16, tag="eb", name=f"eb{b}_{qt}_{h}")
                nc.gpsimd.dma_start(t[:], eb[qt, b, h])
                eb_t[(b, qt, h)] = t

            # ---- constants (wq first: the first projection needs only it) ----
            wq_t = constp.tile([128, DC, 128], F16, tag="wq")
            nc.sync.dma_start(wq_t[:], wq[:])
            indh_t = constp.tile([1, 128], F16, tag="indh")
            nc.sync.dma_start(indh_t[:], indh[:])
            wk_t = constp.tile([128, DC, 128], F16, tag="wk")
            nc.sync.dma_start(wk_t[:], wk[:])
            wv_t = constp.tile([128, DC, 128], F16, tag="wv")
            nc.sync.dma_start(wv_t[:], wv[:])
            wo_t = constp.tile([128, D], F16, tag="wo")
            nc.sync.dma_start(wo_t[:], wo[:])
            identr_t = constp.tile([128, 128], F32R, tag="identr")
            nc.sync.dma_start(identr_t[:], identr[:])

            def emit_pv(b, h, ctx, kp, pt):
                for ki in range(2):
                    kt = 2 * kp + ki
                    nc.tensor.matmul(
                        ctx[:],
                        hvT[b][:, kt, 2 * h : 2 * h + 2, :],
                        pt[:, ki * QTS : (ki + 1) * QTS],
                        start=(kt == 0),
                        stop=(kt == KN - 1),
                    )

            # ---- prologue helpers (used inline for b0, as interleaved chunks for b1)
            hq_sb, hk_sb, hvT, hvf = {}, {}, {}, {}

            def emit_hq(b, get_c):
                hq_ps = [
                    psA.tile([128, 1024], F32, tag="A", name=f"hqps{b}_{i}")
                    for i in range(2)
                ]
                for dc in range(DC):
                    c = get_c(dc)
                    for qt in range(QN):
                        nc.tensor.matmul(
                            hq_ps[qt // 2][:, (qt % 2) * QTS : (qt % 2 + 1) * QTS],
                            wq_t[:, dc, :],
                            c[:, qt * QTS : (qt + 1) * QTS],
                            start=(dc == 0),
                            stop=(dc == DC - 1),
                        )
                hq_sb[b] = hqp.tile([128, L], F16, tag="hq", name=f"hq{b}")
                for i in range(2):
                    nc.vector.tensor_copy(
                        hq_sb[b][:, i * 1024 : (i + 1) * 1024], hq_ps[i][:]
                    )

            def emit_hk(b, get_c, also_hv):
                hk_ps = [
                    psA.tile([128, 1024], F32, tag="A", name=f"hkps{b}_{i}")
                    for i in range(2)
                ]
                if also_hv:
                    hv_ps = [
                        psB.tile([128, QTS], F32, tag="B", name=f"hvps{b}_{i}")
                        for i in range(QN)
                    ]
                for dc in range(DC):
                    c = get_c(dc)
                    for qt in range(QN):
                        nc.tensor.matmul(
                            hk_ps[qt // 2][:, (qt % 2) * QTS : (qt % 2 + 1) * QTS],
                            wk_t[:, dc, :],
                            c[:, qt * QTS : (qt + 1) * QTS],
                            start=(dc == 0),
                            stop=(dc == DC - 1),
                        )
                        if also_hv:
                            nc.tensor.matmul(
                                hv_ps[qt][:],
                                wv_t[:, dc, :],
                                c[:, qt * QTS : (qt + 1) * QTS],
                                start=(dc == 0),
                                stop=(dc == DC - 1),
                            )
                hk_sb[b] = hkp.tile([128, L], F16, tag="hk", name=f"hk{b}")
                for i in range(2):
                    nc.vector.tensor_copy(
                        hk_sb[b][:, i * 1024 : (i + 1) * 1024], hk_ps[i][:]
                    )
                if also_hv:
                    emit_hv_drain(b, hv_ps)

            def emit_hv(b, get_c):
                hv_ps = [
                    psB.tile([128, QTS], F32, tag="B", name=f"hvps{b}_{i}")
                    for i in range(QN)
                ]
                for dc in range(DC):
                    c = get_c(dc)
                    for qt in range(QN):
                        nc.tensor.matmul(
                            hv_ps[qt][:],
                            wv_t[:, dc, :],
                            c[:, qt * QTS : (qt + 1) * QTS],
                            start=(dc == 0),
                            stop=(dc == DC - 1),
                        )
                emit_hv_drain(b, hv_ps)

            def emit_hv_drain(b, hv_ps):
                hvf[b] = hvlp.tile([128, L], F32R, tag="hvl", name=f"hvf{b}")
                for i in range(QN):
                    nc.vector.tensor_copy(
                        hvf[b][:, i * QTS : (i + 1) * QTS], hv_ps[i][:]
                    )

            def emit_hv_t(b):
                # k-major transposed V, per k-tile [v_h0 | ones64 | v_h1 | ones64]
                # so each head's 128-wide stationary slice carries its values
                # plus denominator (all-ones) columns
                hvT[b] = hvtp.tile([128, KN, 4, 64], F16, tag="hvt", name=f"hvt{b}")
                nc.gpsimd.memset(hvT[b][:].bitcast(mybir.dt.uint16), 0x3C00)
                for kt in range(KN):
                    tp = psB.tile([128, 2, 64], F32R, tag="B", name=f"tp{b}_{kt}")
                    nc.tensor.transpose(
                        tp[:], hvf[b][:, kt * 128 : (kt + 1) * 128], identr_t[:]
                    )
                    nc.vector.tensor_copy(hvT[b][:, kt, 0::2, :], tp[:])

            # ---- b0 prologue: stream q/kv chunks, hk+hv share one kv stream ----
            def stream_q0(dc):
                c = stagep.tile([128, L], F16, tag="stage")
                nc.sync.dma_start(c[:], qT[0, dc])
                return c

            def stream_kv0(dc):
                c = stagep.tile([128, L], F16, tag="stage")
                nc.sync.dma_start(c[:], kvT[0, dc])
                return c

            emit_hq(0, stream_q0)
            emit_hk(0, stream_kv0, also_hv=True)
            emit_hv_t(0)

            # ---- b1 inputs staged into persistent tiles; the manual waits
            # keep these bulk DMAs from being scheduled ahead of b0's
            # critical-path staging (they share HBM bandwidth round-robin)
            qb1 = qb1p.tile([128, DC, L], F16, tag="qb1")
            kvb1 = kvb1p.tile([128, DC, L], F16, tag="kvb1")
            with tc.tile_wait_until(0.030):
                nc.sync.dma_start(
                    kvb1[:], kvT[1].rearrange("dc p l -> p dc l")
                )
            with tc.tile_wait_until(0.040):
                nc.sync.dma_start(
                    qb1[:], qT[1].rearrange("dc p l -> p dc l")
                )
            with tc.tile_wait_until(0.025):
                fetch_eb(0)

            # b1 prologue chunks, emitted between the first b0 segments so the
            # score pipeline keeps the Act/Vector engines fed while the PE
            # works through them
            chunks = [
                lambda: emit_hk(1, lambda dc: kvb1[:, dc, :], also_hv=False),
                lambda: emit_hv(1, lambda dc: kvb1[:, dc, :]),
                lambda: emit_hq(1, lambda dc: qb1[:, dc, :]),
                lambda: emit_hv_t(1),
            ]

            # ---- main loop: per (b, q-tile, head) one full k-sweep; PV lags
            # one k-pair behind QK/exp/mult so the in-order PE queue never
            # stalls on the Act/DVE stages
            ctxn_cur = None
            for si, (b, qt, h) in enumerate(segs):
                fetch_eb(si + 1)
                ebt = eb_t.pop((b, qt, h))
                ctx = psB.tile([128, QTS], F32, tag="B", name=f"ctx{b}_{qt}_{h}")
                pending = []
                for kp in range(KPN):
                    sc = psA.tile(
                        [128, 1024], F32, tag="A", name=f"sc{b}_{qt}_{h}_{kp}"
                    )
                    for ki in range(2):
                        kt = 2 * kp + ki
                        nc.tensor.matmul(
                            sc[:, ki * QTS : (ki + 1) * QTS],
                            hk_sb[b][h * DH : (h + 1) * DH, kt * 128 : (kt + 1) * 128],
                            hq_sb[b][h * DH : (h + 1) * DH, qt * QTS : (qt + 1) * QTS],
                            start=True,
                            stop=True,
                        )
                    p0 = p0p.tile([128, 1024], F16, tag="p0")
                    nc.scalar.activation(p0[:], sc[:], EXP)
                    pt = ptp.tile([128, 1024], F16, tag="pt")
                    nc.vector.tensor_tensor(pt[:], p0[:], ebt[:, kp, :], MULT)
                    pending.append((kp, pt))
                    if len(pending) > 2:
                        emit_pv(b, h, ctx, *pending.pop(0))
                for item in pending:
                    emit_pv(b, h, ctx, *item)
                # epilogue for (b, qt, h): normalize into the h-half of ctxn
                if h == 0:
                    ctxn_cur = ctxnp.tile(
                        [128, QTS], F16, tag="ctxn", name=f"ctxn{b}_{qt}"
                    )
                dsb = rcp.tile([1, QTS], F32, tag="dsb")
                nc.vector.tensor_copy(dsb[:], ctx[DH : DH + 1, :])
                rcf = rcp.tile([1, QTS], F32, tag="rcf")
                nc.vector.reciprocal_approx_fast(rcf[:], dsb[:])
                rcr = rcp.tile([1, QTS], F16, tag="rcr")
                nc.vector.tensor_copy(rcr[:], rcf[:])
                bcw = psB.tile([128, QTS], F32, tag="B", name=f"bcw{b}_{qt}_{h}")
                nc.tensor.matmul(bcw[:], indh_t[:], rcr[:], start=True, stop=True)
                bc_sb = rcp.tile([64, QTS], F16, tag="bcsb")
                nc.vector.tensor_copy(bc_sb[:], bcw[0:DH, :])
                nc.vector.tensor_tensor(
                    ctxn_cur[h * DH : (h + 1) * DH, :], ctx[0:DH, :], bc_sb[:], MULT
                )
                if h == 1:
                    # output projection for (b, qt)
                    for qs in range(QN):
                        ob = outp.tile(
                            [128, D], F16, tag="outb", name=f"ob{b}_{qt}_{qs}"
                        )
                        for oh in range(2):
                            op = psB.tile(
                                [128, QTS], F32, tag="B", name=f"op{b}_{qt}_{qs}_{oh}"
                            )
                            nc.tensor.matmul(
                                op[:],
                                ctxn_cur[:, qs * 128 : (qs + 1) * 128],
                                wo_t[:, oh * QTS : (oh + 1) * QTS],
                                start=True,
                                stop=True,
                            )
                            if oh == 0:
                                nc.vector.tensor_copy(
                                    ob[:, oh * QTS : (oh + 1) * QTS], op[:]
                                )
                            else:
                                nc.scalar.copy(
                                    ob[:, oh * QTS : (oh + 1) * QTS], op[:]
                                )
                        r0 = qt * QTS + qs * 128
                        nc.sync.dma_start(out[b, r0 : r0 + 128, :], ob[:])
                if si < len(chunks):
                    chunks[si]()

    nc.compile()
    _CACHE["nc"] = nc
    return nc


def _prep_core(core, Wq, Wk, Wv, Wo, shared):
    """Per-core input map. `shared` holds core-independent packed arrays."""
    h0 = core * HPC
    rows = slice(h0 * DH, (h0 + HPC) * DH)

    def packw(w, scale=1.0):
        return np.ascontiguousarray(
            (w[rows].T * scale).reshape(DC, 128, 128).transpose(1, 0, 2)
        ).astype(np.float16)

    # eb[qt, b, hl, p, kp, ki*512+qf] = mask[b, q, k] * exp(pb[h, q, k])
    # with q = qt*512+qf, k = (2*kp+ki)*128+p  (fp16 bit arithmetic in uint16)
    expT, maskT = shared["expT"], shared["maskT"]
    ebc = np.empty((QN, B, HPC, 128, KPN, 1024), np.uint16)
    for qt in range(QN):
        for b in range(B):
            for hl in range(HPC):
                np.multiply(expT[h0 + hl, qt], maskT[b, qt], out=ebc[qt, b, hl])
    return {
        "qT": shared["qT"],
        "kvT": shared["kvT"],
        "indh": shared["indh"],
        "identr": shared["identr"],
        "wq": packw(Wq, 1.0 / np.sqrt(DH)),
        "wk": packw(Wk),
        "wv": packw(Wv),
        "wo": np.ascontiguousarray(Wo[:, rows].T).astype(np.float16),
        "eb": ebc.view(np.float16),
    }


def _prep_shared(query, key_value, mask, position_bias):
    qTp = np.ascontiguousarray(
        query.reshape(B, L, DC, 128).transpose(0, 2, 3, 1)
    ).astype(np.float16)
    kvTp = np.ascontiguousarray(
        key_value.reshape(B, L, DC, 128).transpose(0, 2, 3, 1)
    ).astype(np.float16)
    # [h, q, k] -> [h, qt, p, kp, ki, qf] (fp16 bits as uint16)
    expT = (
        np.ascontiguousarray(
            np.exp(position_bias.astype(np.float32))
            .astype(np.float16)
            .reshape(H, QN, QTS, KPN, 2, 128)
            .transpose(0, 1, 5, 3, 4, 2)
        )
        .view(np.uint16)
        .reshape(H, QN, 128, KPN, 1024)
    )
    maskT = np.ascontiguousarray(
        (np.asarray(mask, dtype=bool))
        .astype(np.uint16)
        .reshape(B, QN, QTS, KPN, 2, 128)
        .transpose(0, 1, 5, 3, 4, 2)
    ).reshape(B, QN, 128, KPN, 1024)
    indh = np.where(np.arange(128) < 64, 1.0, 0.0).astype(np.float16)[None, :]
    return {
        "qT": qTp,
        "kvT": kvTp,
        "expT": expT,
        "maskT": maskT,
        "indh": np.ascontiguousarray(indh),
        "identr": np.eye(128, dtype=np.float32),
    }


def kernel(query, key_value, mask, position_bias, Wq, Wk, Wv, Wo, _trace=False):
    query = np.asarray(query, dtype=np.float32)
    key_value = np.asarray(key_value, dtype=np.float32)
    mask = np.asarray(mask)
    position_bias = np.asarray(position_bias, dtype=np.float32)
    Wq = np.asarray(Wq, dtype=np.float32)
    Wk = np.asarray(Wk, dtype=np.float32)
    Wv = np.asarray(Wv, dtype=np.float32)
    Wo = np.asarray(Wo, dtype=np.float32)

    nc = _build()
    shared = _prep_shared(query, key_value, mask, position_bias)
    in_maps = [_prep_core(c, Wq, Wk, Wv, Wo, shared) for c in range(N_CORES)]
    res = run_bass_kernel_spmd(nc, in_maps, list(range(N_CORES)), trace=_trace)
    _CACHE["last_result"] = res
    acc = res.results[0]["out"].astype(np.float64)
    for c in range(1, N_CORES):
        acc += res.results[c]["out"]
    return acc.astype(np.float32)


# revision 16
# speedup vs baseline: 1.0336x; 1.0228x over previous
"""CPM3 attention kernel for 8 trn2 NeuronCores.

Sharding: tensor-parallel over heads (2 heads/core x both batches).
Device computes per-core partial outputs (Wo row-sharded); host sums.

Structure (v2):
- mask+bias are folded on the host into one multiplicative fp16 table
  eb = mask * exp(position_bias), using exp(s + pb) * mask == exp(s) * eb.
  This removes the per-tile DVE mask/bias fuse and the PSUM identity-matmul
  injection of the additive design (big PE + DVE savings).
- scores are computed transposed [k, q] so the softmax needs no
  partition-dim reductions; denominators come free from an all-ones block
  appended to the transposed-V stationary (one extra PSUM partition).
- per (b, head): the k-sweep runs in 8 chunks of 2 k-tiles: 2 QK matmuls
  into one 2-bank PSUM tile, a single EXP (Act engine) over [128,1024],
  a single fp16 multiply by eb (DVE 2x_1P mode), then 2 PV matmuls.
- hv is transposed k-major via the DMA xbar (SBUF->SBUF), not the PE.
- fp16 operands for all matmuls (2-byte weights use the PE background
  weight-load path and halve HBM traffic); PSUM accumulation stays fp32.
"""

import sys

sys.path.insert(0, "/opt/trn_rl_repo")

import numpy as np

import concourse.bass as bass
import concourse.bacc as bacc
import concourse.tile as tile
import concourse.mybir as mybir
from concourse.bass_utils import run_bass_kernel_spmd

B, L, D, H, DH = 2, 2048, 1024, 16, 64
N_CORES = 8
HPC = H // N_CORES  # heads per core = 2
QTS = 512  # q tile size
QN = L // QTS  # 4
KN = L // 128  # 16 k-tiles
KPN = KN // 2  # 8 k-pairs (2 k-tiles share one 2-bank PSUM score tile)
DC = D // 128  # 8 contraction chunks
HVW = 256  # transposed-V columns per k-tile: [v_h0 | ones64 | v_h1 | ones64]

F32 = mybir.dt.float32
F32R = mybir.dt.float32r
F16 = mybir.dt.float16

_CACHE: dict = {}


def _build():
    if "nc" in _CACHE:
        return _CACHE["nc"]
    nc = bacc.Bacc("TRN2", target_bir_lowering=False, debug=False, num_devices=N_CORES)

    qT = nc.dram_tensor("qT", [B, DC, 128, L], F16, kind="ExternalInput").ap()
    kvT = nc.dram_tensor("kvT", [B, DC, 128, L], F16, kind="ExternalInput").ap()
    wq = nc.dram_tensor("wq", [128, DC, 128], F16, kind="ExternalInput").ap()
    wk = nc.dram_tensor("wk", [128, DC, 128], F16, kind="ExternalInput").ap()
    wv = nc.dram_tensor("wv", [128, DC, 128], F16, kind="ExternalInput").ap()
    wo = nc.dram_tensor("wo", [128, D], F16, kind="ExternalInput").ap()
    eb = nc.dram_tensor(
        "eb", [QN, B, HPC, 128, KPN, 1024], F16, kind="ExternalInput"
    ).ap()
    indh = nc.dram_tensor("indh", [1, 128], F16, kind="ExternalInput").ap()
    identr = nc.dram_tensor("identr", [128, 128], F32R, kind="ExternalInput").ap()
    out = nc.dram_tensor("out", [B, L, D], F16, kind="ExternalOutput").ap()

    EXP = mybir.ActivationFunctionType.Exp
    MULT = mybir.AluOpType.mult

    with tile.TileContext(nc) as tc:
        with (
            tc.tile_pool(name="const", bufs=1) as constp,
            tc.tile_pool(name="stage", bufs=2) as stagep,
            tc.tile_pool(name="qb1", bufs=1) as qb1p,
            tc.tile_pool(name="kvb1", bufs=1) as kvb1p,
            tc.tile_pool(name="hq", bufs=2) as hqp,
            tc.tile_pool(name="hk", bufs=2) as hkp,
            tc.tile_pool(name="hvl", bufs=1) as hvlp,
            tc.tile_pool(name="hvt", bufs=2) as hvtp,
            tc.tile_pool(name="ebp", bufs=2) as ebp,
            tc.tile_pool(name="p0", bufs=3) as p0p,
            tc.tile_pool(name="pt", bufs=5) as ptp,
            tc.tile_pool(name="ctxn", bufs=2) as ctxnp,
            tc.tile_pool(name="rc", bufs=4) as rcp,
            tc.tile_pool(name="outb", bufs=2) as outp,
            tc.tile_pool(name="psA", bufs=2, space=bass.MemorySpace.PSUM) as psA,
            tc.tile_pool(name="psB", bufs=4, space=bass.MemorySpace.PSUM) as psB,
        ):
            # ---- eb prefetch for the first segments (segments are b-major) ----
            segs = [
                (b, qt, h) for b in range(B) for qt in range(QN) for h in range(HPC)
            ]
            eb_t = {}

            def fetch_eb(i):
                if i >= len(segs):
                    return
                b, qt, h = segs[i]
                t = ebp.tile([128, KPN, 1024], F16, tag="eb", name=f"eb{b}_{qt}_{h}")
                nc.gpsimd.dma_start(t[:], eb[qt, b, h])
                eb_t[(b, qt, h)] = t

            # ---- constants (wq first: the first projection needs only it) ----
            wq_t = constp.tile([128, DC, 128], F16, tag="wq")
            nc.sync.dma_start(wq_t[:], wq[:])
            indh_t = constp.tile([1, 128], F16, tag="indh")
            nc.sync.dma_start(indh_t[:], indh[:])
            wk_t = constp.tile([128, DC, 128], F16, tag="wk")
            nc.sync.dma_start(wk_t[:], wk[:])
            wv_t = constp.tile([128, DC, 128], F16, tag="wv")
            nc.sync.dma_start(wv_t[:], wv[:])
            wo_t = constp.tile([128, D], F16, tag="wo")
            nc.sync.dma_start(wo_t[:], wo[:])
            identr_t = constp.tile([128, 128], F32R, tag="identr")
            nc.sync.dma_start(identr_t[:], identr[:])

            def emit_pv(b, h, ctx, kp, pt):
                for ki in range(2):
                    kt = 2 * kp + ki
                    nc.tensor.matmul(
                        ctx[:],
                        hvT[b][:, kt, 2 * h : 2 * h + 2, :],
                        pt[:, ki * QTS : (ki + 1) * QTS],
                        start=(kt == 0),
                        stop=(kt == KN - 1),
                    )

            # ---- prologue helpers (used inline for b0, as interleaved chunks for b1)
            hq_sb, hk_sb, hvT, hvf = {}, {}, {}, {}

            def emit_hq(b, get_c):
                hq_ps = [
                    psA.tile([128, 1024], F32, tag="A", name=f"hqps{b}_{i}")
                    for i in range(2)
                ]
                for dc in range(DC):
                    c = get_c(dc)
                    for qt in range(QN):
                        nc.tensor.matmul(
                            hq_ps[qt // 2][:, (qt % 2) * QTS : (qt % 2 + 1) * QTS],
                            wq_t[:, dc, :],
                            c[:, qt * QTS : (qt + 1) * QTS],
                            start=(dc == 0),
                            stop=(dc == DC - 1),
                        )
                hq_sb[b] = hqp.tile([128, L], F16, tag="hq", name=f"hq{b}")
                for i in range(2):
                    nc.vector.tensor_copy(
                        hq_sb[b][:, i * 1024 : (i + 1) * 1024], hq_ps[i][:]
                    )

            def emit_hk(b, get_c, also_hv):
                hk_ps = [
                    psA.tile([128, 1024], F32, tag="A", name=f"hkps{b}_{i}")
                    for i in range(2)
                ]
                if also_hv:
                    hv_ps = [
                        psB.tile([128, QTS], F32, tag="B", name=f"hvps{b}_{i}")
                        for i in range(QN)
                    ]
                for dc in range(DC):
                    c = get_c(dc)
                    for qt in range(QN):
                        nc.tensor.matmul(
                            hk_ps[qt // 2][:, (qt % 2) * QTS : (qt % 2 + 1) * QTS],
                            wk_t[:, dc, :],
                            c[:, qt * QTS : (qt + 1) * QTS],
                            start=(dc == 0),
                            stop=(dc == DC - 1),
                        )
                        if also_hv:
                            nc.tensor.matmul(
                                hv_ps[qt][:],
                                wv_t[:, dc, :],
                                c[:, qt * QTS : (qt + 1) * QTS],
                                start=(dc == 0),
                                stop=(dc == DC - 1),
                            )
                hk_sb[b] = hkp.tile([128, L], F16, tag="hk", name=f"hk{b}")
                for i in range(2):
                    nc.vector.tensor_copy(
                        hk_sb[b][:, i * 1024 : (i + 1) * 1024], hk_ps[i][:]
                    )
                if also_hv:
                    emit_hv_drain(b, hv_ps)

            def emit_hv(b, get_c):
                hv_ps = [
                    psB.tile([128, QTS], F32, tag="B", name=f"hvps{b}_{i}")
                    for i in range(QN)
                ]
                for dc in range(DC):
                    c = get_c(dc)
                    for qt in range(QN):
                        nc.tensor.matmul(
                            hv_ps[qt][:],
                            wv_t[:, dc, :],
                            c[:, qt * QTS : (qt + 1) * QTS],
                            start=(dc == 0),
                            stop=(dc == DC - 1),
                        )
                emit_hv_drain(b, hv_ps)

            def emit_hv_drain(b, hv_ps):
                hvf[b] = hvlp.tile([128, L], F32R, tag="hvl", name=f"hvf{b}")
                for i in range(QN):
                    nc.vector.tensor_copy(
                        hvf[b][:, i * QTS : (i + 1) * QTS], hv_ps[i][:]
                    )

            def emit_hv_t(b):
                # k-major transposed V, per k-tile [v_h0 | ones64 | v_h1 | ones64]
                # so each head's 128-wide stationary slice carries its values
                # plus denominator (all-ones) columns
                hvT[b] = hvtp.tile([128, KN, 4, 64], F16, tag="hvt", name=f"hvt{b}")
                nc.gpsimd.memset(hvT[b][:].bitcast(mybir.dt.uint16), 0x3C00)
                for kt in range(KN):
                    tp = psB.tile([128, 2, 64], F32R, tag="B", name=f"tp{b}_{kt}")
                    nc.tensor.transpose(
                        tp[:], hvf[b][:, kt * 128 : (kt + 1) * 128], identr_t[:]
                    )
                    nc.vector.tensor_copy(hvT[b][:, kt, 0::2, :], tp[:])

            # ---- b0 prologue: stream q/kv chunks, hk+hv share one kv stream ----
            def stream_q0(dc):
                c = stagep.tile([128, L], F16, tag="stage")
                nc.sync.dma_start(c[:], qT[0, dc])
                return c

            def stream_kv0(dc):
                c = stagep.tile([128, L], F16, tag="stage")
                nc.sync.dma_start(c[:], kvT[0, dc])
                return c

            emit_hq(0, stream_q0)
            emit_hk(0, stream_kv0, also_hv=True)
            emit_hv_t(0)

            # ---- b1 inputs staged into persistent tiles; the manual waits
            # keep these bulk DMAs from being scheduled ahead of b0's
            # critical-path staging (they share HBM bandwidth round-robin)
            qb1 = qb1p.tile([128, DC, L], F16, tag="qb1")
            kvb1 = kvb1p.tile([128, DC, L], F16, tag="kvb1")
            with tc.tile_wait_until(0.055):
                nc.sync.dma_start(
                    kvb1[:], kvT[1].rearrange("dc p l -> p dc l")
                )
            with tc.tile_wait_until(0.065):
                nc.sync.dma_start(
                    qb1[:], qT[1].rearrange("dc p l -> p dc l")
                )
            with tc.tile_wait_until(0.045):
                fetch_eb(0)

            # b1 prologue chunks, emitted between the first b0 segments so the
            # score pipeline keeps the Act/Vector engines fed while the PE
            # works through them
            chunks = [
                lambda: emit_hk(1, lambda dc: kvb1[:, dc, :], also_hv=False),
                lambda: emit_hv(1, lambda dc: kvb1[:, dc, :]),
                lambda: emit_hq(1, lambda dc: qb1[:, dc, :]),
                lambda: emit_hv_t(1),
            ]

            # ---- main loop: per (b, q-tile, head) one full k-sweep.
            # PV lags two k-pairs behind QK/exp/mult, and both the final two
            # PV pairs and the whole normalize/out-project epilogue of each
            # segment are deferred into the next segment's first k-pairs, so
            # the in-order PE and Act queues never drain at seg boundaries.
            state = {"ctxn": None}

            def make_epi(b, qt, h, ctx):
                def epi():
                    if h == 0:
                        state["ctxn"] = ctxnp.tile(
                            [128, QTS], F16, tag="ctxn", name=f"ctxn{b}_{qt}"
                        )
                    ctxn_cur = state["ctxn"]
                    dsb = rcp.tile([1, QTS], F32, tag="dsb")
                    nc.vector.tensor_copy(dsb[:], ctx[DH : DH + 1, :])
                    rcf = rcp.tile([1, QTS], F32, tag="rcf")
                    nc.vector.reciprocal_approx_fast(rcf[:], dsb[:])
                    rcr = rcp.tile([1, QTS], F16, tag="rcr")
                    nc.vector.tensor_copy(rcr[:], rcf[:])
                    bcw = psB.tile([128, QTS], F32, tag="B", name=f"bcw{b}_{qt}_{h}")
                    nc.tensor.matmul(bcw[:], indh_t[:], rcr[:], start=True, stop=True)
                    bc_sb = rcp.tile([64, QTS], F16, tag="bcsb")
                    nc.vector.tensor_copy(bc_sb[:], bcw[0:DH, :])
                    nc.vector.tensor_tensor(
                        ctxn_cur[h * DH : (h + 1) * DH, :], ctx[0:DH, :], bc_sb[:], MULT
                    )
                    if h == 1:
                        for qs in range(QN):
                            ob = outp.tile(
                                [128, D], F16, tag="outb", name=f"ob{b}_{qt}_{qs}"
                            )
                            for oh in range(2):
                                op = psB.tile(
                                    [128, QTS],
                                    F32,
                                    tag="B",
                                    name=f"op{b}_{qt}_{qs}_{oh}",
                                )
                                nc.tensor.matmul(
                                    op[:],
                                    ctxn_cur[:, qs * 128 : (qs + 1) * 128],
                                    wo_t[:, oh * QTS : (oh + 1) * QTS],
                                    start=True,
                                    stop=True,
                                )
                                if oh == 0:
                                    nc.vector.tensor_copy(
                                        ob[:, oh * QTS : (oh + 1) * QTS], op[:]
                                    )
                                else:
                                    nc.scalar.copy(
                                        ob[:, oh * QTS : (oh + 1) * QTS], op[:]
                                    )
                            r0 = qt * QTS + qs * 128
                            nc.sync.dma_start(out[b, r0 : r0 + 128, :], ob[:])

                return epi

            carry = []  # PV pairs of the previous segment still to emit
            pending_epi = None
            for si, (b, qt, h) in enumerate(segs):
                fetch_eb(si + 1)
                ebt = eb_t.pop((b, qt, h))
                ctx = psB.tile([128, QTS], F32, tag="B", name=f"ctx{b}_{qt}_{h}")
                pending = []
                for kp in range(KPN):
                    sc = psA.tile(
                        [128, 1024], F32, tag="A", name=f"sc{b}_{qt}_{h}_{kp}"
                    )
                    for ki in range(2):
                        kt = 2 * kp + ki
                        nc.tensor.matmul(
                            sc[:, ki * QTS : (ki + 1) * QTS],
                            hk_sb[b][h * DH : (h + 1) * DH, kt * 128 : (kt + 1) * 128],
                            hq_sb[b][h * DH : (h + 1) * DH, qt * QTS : (qt + 1) * QTS],
                            start=True,
                            stop=True,
                        )
                    p0 = p0p.tile([128, 1024], F16, tag="p0")
                    nc.scalar.activation(p0[:], sc[:], EXP)
                    pt = ptp.tile([128, 1024], F16, tag="pt")
                    nc.vector.tensor_tensor(pt[:], p0[:], ebt[:, kp, :], MULT)
                    if carry:
                        carry.pop(0)()
                    elif pending_epi is not None:
                        pending_epi()
                        pending_epi = None
                    pending.append((kp, pt))
                    if kp >= 2:
                        emit_pv(b, h, ctx, *pending.pop(0))
                # defer the last two PV pairs and the epilogue into the next seg
                def mk_pv(bb, hh, cc, item):
                    return lambda: emit_pv(bb, hh, cc, *item)

                carry = [mk_pv(b, h, ctx, item) for item in pending]
                if pending_epi is not None:
                    pending_epi()
                pending_epi = make_epi(b, qt, h, ctx)
                if si < len(chunks):
                    chunks[si]()
            for fn in carry:
                fn()
            pending_epi()

    nc.compile()
    _CACHE["nc"] = nc
    return nc


def _prep_core(core, Wq, Wk, Wv, Wo, shared):
    """Per-core input map. `shared` holds core-independent packed arrays."""
    h0 = core * HPC
    rows = slice(h0 * DH, (h0 + HPC) * DH)

    def packw(w, scale=1.0):
        return np.ascontiguousarray(
            (w[rows].T * scale).reshape(DC, 128, 128).transpose(1, 0, 2)
        ).astype(np.float16)

    # eb[qt, b, hl, p, kp, ki*512+qf] = mask[b, q, k] * exp(pb[h, q, k])
    # with q = qt*512+qf, k = (2*kp+ki)*128+p  (fp16 bit arithmetic in uint16)
    expT, maskT = shared["expT"], shared["maskT"]
    ebc = np.empty((QN, B, HPC, 128, KPN, 1024), np.uint16)
    for qt in range(QN):
        for b in range(B):
            for hl in range(HPC):
                np.multiply(expT[h0 + hl, qt], maskT[b, qt], out=ebc[qt, b, hl])
    return {
        "qT": shared["qT"],
        "kvT": shared["kvT"],
        "indh": shared["indh"],
        "identr": shared["identr"],
        "wq": packw(Wq, 1.0 / np.sqrt(DH)),
        "wk": packw(Wk),
        "wv": packw(Wv),
        "wo": np.ascontiguousarray(Wo[:, rows].T).astype(np.float16),
        "eb": ebc.view(np.float16),
    }


def _prep_shared(query, key_value, mask, position_bias):
    qTp = np.ascontiguousarray(
        query.reshape(B, L, DC, 128).transpose(0, 2, 3, 1)
    ).astype(np.float16)
    kvTp = np.ascontiguousarray(
        key_value.reshape(B, L, DC, 128).transpose(0, 2, 3, 1)
    ).astype(np.float16)
    # [h, q, k] -> [h, qt, p, kp, ki, qf] (fp16 bits as uint16)
    expT = (
        np.ascontiguousarray(
            np.exp(position_bias.astype(np.float32))
            .astype(np.float16)
            .reshape(H, QN, QTS, KPN, 2, 128)
            .transpose(0, 1, 5, 3, 4, 2)
        )
        .view(np.uint16)
        .reshape(H, QN, 128, KPN, 1024)
    )
    maskT = np.ascontiguousarray(
        (np.asarray(mask, dtype=bool))
        .astype(np.uint16)
        .reshape(B, QN, QTS, KPN, 2, 128)
        .transpose(0, 1, 5, 3, 4, 2)
    ).reshape(B, QN, 128, KPN, 1024)
    indh = np.where(np.arange(128) < 64, 1.0, 0.0).astype(np.float16)[None, :]
    return {
        "qT": qTp,
        "kvT": kvTp,
        "expT": expT,
        "maskT": maskT,
        "indh": np.ascontiguousarray(indh),
        "identr": np.eye(128, dtype=np.float32),
    }


def kernel(query, key_value, mask, position_bias, Wq, Wk, Wv, Wo, _trace=False):
    query = np.asarray(query, dtype=np.float32)
    key_value = np.asarray(key_value, dtype=np.float32)
    mask = np.asarray(mask)
    position_bias = np.asarray(position_bias, dtype=np.float32)
    Wq = np.asarray(Wq, dtype=np.float32)
    Wk = np.asarray(Wk, dtype=np.float32)
    Wv = np.asarray(Wv, dtype=np.float32)
    Wo = np.asarray(Wo, dtype=np.float32)

    nc = _build()
    shared = _prep_shared(query, key_value, mask, position_bias)
    in_maps = [_prep_core(c, Wq, Wk, Wv, Wo, shared) for c in range(N_CORES)]
    res = run_bass_kernel_spmd(nc, in_maps, list(range(N_CORES)), trace=_trace)
    _CACHE["last_result"] = res
    acc = res.results[0]["out"].astype(np.float64)
    for c in range(1, N_CORES):
        acc += res.results[c]["out"]
    return acc.astype(np.float32)


# revision 17
# speedup vs baseline: 1.0583x; 1.0238x over previous
"""CPM3 attention kernel for 8 trn2 NeuronCores.

Sharding: tensor-parallel over heads (2 heads/core x both batches).
Device computes per-core partial outputs (Wo row-sharded); host sums.

Structure (v2):
- mask+bias are folded on the host into one multiplicative fp16 table
  eb = mask * exp(position_bias), using exp(s + pb) * mask == exp(s) * eb.
  This removes the per-tile DVE mask/bias fuse and the PSUM identity-matmul
  injection of the additive design (big PE + DVE savings).
- scores are computed transposed [k, q] so the softmax needs no
  partition-dim reductions; denominators come free from an all-ones block
  appended to the transposed-V stationary (one extra PSUM partition).
- per (b, head): the k-sweep runs in 8 chunks of 2 k-tiles: 2 QK matmuls
  into one 2-bank PSUM tile, a single EXP (Act engine) over [128,1024],
  a single fp16 multiply by eb (DVE 2x_1P mode), then 2 PV matmuls.
- hv is transposed k-major via the DMA xbar (SBUF->SBUF), not the PE.
- fp16 operands for all matmuls (2-byte weights use the PE background
  weight-load path and halve HBM traffic); PSUM accumulation stays fp32.
"""

import sys

sys.path.insert(0, "/opt/trn_rl_repo")

import numpy as np

import concourse.bass as bass
import concourse.bacc as bacc
import concourse.tile as tile
import concourse.mybir as mybir
from concourse.bass_utils import run_bass_kernel_spmd

B, L, D, H, DH = 2, 2048, 1024, 16, 64
N_CORES = 8
HPC = H // N_CORES  # heads per core = 2
QTS = 512  # q tile size
QN = L // QTS  # 4
KN = L // 128  # 16 k-tiles
KPN = KN // 2  # 8 k-pairs (2 k-tiles share one 2-bank PSUM score tile)
DC = D // 128  # 8 contraction chunks
HVW = 256  # transposed-V columns per k-tile: [v_h0 | ones64 | v_h1 | ones64]

F32 = mybir.dt.float32
F32R = mybir.dt.float32r
F16 = mybir.dt.float16

_CACHE: dict = {}


def _build():
    if "nc" in _CACHE:
        return _CACHE["nc"]
    nc = bacc.Bacc("TRN2", target_bir_lowering=False, debug=False, num_devices=N_CORES)

    qT = nc.dram_tensor("qT", [B, DC, 128, L], F16, kind="ExternalInput").ap()
    kvT = nc.dram_tensor("kvT", [B, DC, 128, L], F16, kind="ExternalInput").ap()
    wq = nc.dram_tensor("wq", [128, DC, 128], F16, kind="ExternalInput").ap()
    wk = nc.dram_tensor("wk", [128, DC, 128], F16, kind="ExternalInput").ap()
    wv = nc.dram_tensor("wv", [128, DC, 128], F16, kind="ExternalInput").ap()
    wo = nc.dram_tensor("wo", [128, D], F16, kind="ExternalInput").ap()
    eb = nc.dram_tensor(
        "eb", [QN, B, HPC, 128, KPN, 1024], F16, kind="ExternalInput"
    ).ap()
    indh = nc.dram_tensor("indh", [1, 128], F16, kind="ExternalInput").ap()
    identr = nc.dram_tensor("identr", [128, 128], F32R, kind="ExternalInput").ap()
    out = nc.dram_tensor("out", [B, L, D], F16, kind="ExternalOutput").ap()

    EXP = mybir.ActivationFunctionType.Exp
    MULT = mybir.AluOpType.mult

    with tile.TileContext(nc) as tc:
        with (
            tc.tile_pool(name="const", bufs=1) as constp,
            tc.tile_pool(name="stage", bufs=2) as stagep,
            tc.tile_pool(name="qb1", bufs=1) as qb1p,
            tc.tile_pool(name="kvb1", bufs=1) as kvb1p,
            tc.tile_pool(name="hq", bufs=2) as hqp,
            tc.tile_pool(name="hk", bufs=2) as hkp,
            tc.tile_pool(name="hvl", bufs=1) as hvlp,
            tc.tile_pool(name="hvt", bufs=2) as hvtp,
            tc.tile_pool(name="ebp", bufs=2) as ebp,
            tc.tile_pool(name="p0", bufs=3) as p0p,
            tc.tile_pool(name="pt", bufs=5) as ptp,
            tc.tile_pool(name="ctxn", bufs=2) as ctxnp,
            tc.tile_pool(name="rc", bufs=4) as rcp,
            tc.tile_pool(name="outb", bufs=3) as outp,
            tc.tile_pool(name="psA", bufs=2, space=bass.MemorySpace.PSUM) as psA,
            tc.tile_pool(name="psB", bufs=4, space=bass.MemorySpace.PSUM) as psB,
        ):
            # ---- eb prefetch for the first segments (segments are b-major) ----
            segs = [
                (b, qt, h) for b in range(B) for qt in range(QN) for h in range(HPC)
            ]
            eb_t = {}

            def fetch_eb(i):
                if i >= len(segs):
                    return
                b, qt, h = segs[i]
                t = ebp.tile([128, KPN, 1024], F16, tag="eb", name=f"eb{b}_{qt}_{h}")
                nc.gpsimd.dma_start(t[:], eb[qt, b, h])
                eb_t[(b, qt, h)] = t

            # ---- constants (wq first: the first projection needs only it) ----
            wq_t = constp.tile([128, DC, 128], F16, tag="wq")
            nc.sync.dma_start(wq_t[:], wq[:])
            indh_t = constp.tile([1, 128], F16, tag="indh")
            nc.sync.dma_start(indh_t[:], indh[:])
            wk_t = constp.tile([128, DC, 128], F16, tag="wk")
            nc.sync.dma_start(wk_t[:], wk[:])
            wv_t = constp.tile([128, DC, 128], F16, tag="wv")
            nc.sync.dma_start(wv_t[:], wv[:])
            wo_t = constp.tile([128, D], F16, tag="wo")
            nc.sync.dma_start(wo_t[:], wo[:])
            identr_t = constp.tile([128, 128], F32R, tag="identr")
            nc.sync.dma_start(identr_t[:], identr[:])

            def emit_pv(b, h, ctx, kp, pt):
                for ki in range(2):
                    kt = 2 * kp + ki
                    nc.tensor.matmul(
                        ctx[:],
                        hvT[b][:, kt, 2 * h : 2 * h + 2, :],
                        pt[:, ki * QTS : (ki + 1) * QTS],
                        start=(kt == 0),
                        stop=(kt == KN - 1),
                    )

            # ---- prologue helpers (used inline for b0, as interleaved chunks for b1)
            hq_sb, hk_sb, hvT, hvf = {}, {}, {}, {}

            def emit_hq(b, get_c):
                hq_ps = [
                    psA.tile([128, 1024], F32, tag="A", name=f"hqps{b}_{i}")
                    for i in range(2)
                ]
                for dc in range(DC):
                    c = get_c(dc)
                    for qt in range(QN):
                        nc.tensor.matmul(
                            hq_ps[qt // 2][:, (qt % 2) * QTS : (qt % 2 + 1) * QTS],
                            wq_t[:, dc, :],
                            c[:, qt * QTS : (qt + 1) * QTS],
                            start=(dc == 0),
                            stop=(dc == DC - 1),
                        )
                hq_sb[b] = hqp.tile([128, L], F16, tag="hq", name=f"hq{b}")
                for i in range(2):
                    nc.vector.tensor_copy(
                        hq_sb[b][:, i * 1024 : (i + 1) * 1024], hq_ps[i][:]
                    )

            def emit_hk(b, get_c, also_hv):
                hk_ps = [
                    psA.tile([128, 1024], F32, tag="A", name=f"hkps{b}_{i}")
                    for i in range(2)
                ]
                if also_hv:
                    hv_ps = [
                        psB.tile([128, QTS], F32, tag="B", name=f"hvps{b}_{i}")
                        for i in range(QN)
                    ]
                for dc in range(DC):
                    c = get_c(dc)
                    for qt in range(QN):
                        nc.tensor.matmul(
                            hk_ps[qt // 2][:, (qt % 2) * QTS : (qt % 2 + 1) * QTS],
                            wk_t[:, dc, :],
                            c[:, qt * QTS : (qt + 1) * QTS],
                            start=(dc == 0),
                            stop=(dc == DC - 1),
                        )
                        if also_hv:
                            nc.tensor.matmul(
                                hv_ps[qt][:],
                                wv_t[:, dc, :],
                                c[:, qt * QTS : (qt + 1) * QTS],
                                start=(dc == 0),
                                stop=(dc == DC - 1),
                            )
                hk_sb[b] = hkp.tile([128, L], F16, tag="hk", name=f"hk{b}")
                for i in range(2):
                    nc.vector.tensor_copy(
                        hk_sb[b][:, i * 1024 : (i + 1) * 1024], hk_ps[i][:]
                    )
                if also_hv:
                    emit_hv_drain(b, hv_ps)

            def emit_hv(b, get_c):
                hv_ps = [
                    psB.tile([128, QTS], F32, tag="B", name=f"hvps{b}_{i}")
                    for i in range(QN)
                ]
                for dc in range(DC):
                    c = get_c(dc)
                    for qt in range(QN):
                        nc.tensor.matmul(
                            hv_ps[qt][:],
                            wv_t[:, dc, :],
                            c[:, qt * QTS : (qt + 1) * QTS],
                            start=(dc == 0),
                            stop=(dc == DC - 1),
                        )
                emit_hv_drain(b, hv_ps)

            def emit_hv_drain(b, hv_ps):
                hvf[b] = hvlp.tile([128, L], F32R, tag="hvl", name=f"hvf{b}")
                for i in range(QN):
                    nc.vector.tensor_copy(
                        hvf[b][:, i * QTS : (i + 1) * QTS], hv_ps[i][:]
                    )

            def emit_hv_t(b):
                # k-major transposed V, per k-tile [v_h0 | ones64 | v_h1 | ones64]
                # so each head's 128-wide stationary slice carries its values
                # plus denominator (all-ones) columns
                hvT[b] = hvtp.tile([128, KN, 4, 64], F16, tag="hvt", name=f"hvt{b}")
                nc.gpsimd.memset(hvT[b][:].bitcast(mybir.dt.uint16), 0x3C00)
                for kt in range(KN):
                    tp = psB.tile([128, 2, 64], F32R, tag="B", name=f"tp{b}_{kt}")
                    nc.tensor.transpose(
                        tp[:], hvf[b][:, kt * 128 : (kt + 1) * 128], identr_t[:]
                    )
                    nc.vector.tensor_copy(hvT[b][:, kt, 0::2, :], tp[:])

            # ---- b0 prologue: stream q/kv chunks, hk+hv share one kv stream ----
            def stream_q0(dc):
                c = stagep.tile([128, L], F16, tag="stage")
                nc.sync.dma_start(c[:], qT[0, dc])
                return c

            def stream_kv0(dc):
                c = stagep.tile([128, L], F16, tag="stage")
                nc.sync.dma_start(c[:], kvT[0, dc])
                return c

            emit_hq(0, stream_q0)
            emit_hk(0, stream_kv0, also_hv=True)
            emit_hv_t(0)

            # ---- b1 inputs staged into persistent tiles; the manual waits
            # keep these bulk DMAs from being scheduled ahead of b0's
            # critical-path staging (they share HBM bandwidth round-robin)
            qb1 = qb1p.tile([128, DC, L], F16, tag="qb1")
            kvb1 = kvb1p.tile([128, DC, L], F16, tag="kvb1")
            with tc.tile_wait_until(0.055):
                nc.sync.dma_start(
                    kvb1[:], kvT[1].rearrange("dc p l -> p dc l")
                )
            with tc.tile_wait_until(0.065):
                nc.sync.dma_start(
                    qb1[:], qT[1].rearrange("dc p l -> p dc l")
                )
            with tc.tile_wait_until(0.045):
                fetch_eb(0)

            # b1 prologue chunks, emitted between the first b0 segments so the
            # score pipeline keeps the Act/Vector engines fed while the PE
            # works through them
            chunks = [
                lambda: emit_hk(1, lambda dc: kvb1[:, dc, :], also_hv=False),
                lambda: emit_hv(1, lambda dc: kvb1[:, dc, :]),
                lambda: emit_hq(1, lambda dc: qb1[:, dc, :]),
                lambda: emit_hv_t(1),
            ]

            # ---- main loop: per (b, q-tile, head) one full k-sweep.
            # PV lags two k-pairs behind QK/exp/mult, and both the final two
            # PV pairs and the whole normalize/out-project epilogue of each
            # segment are deferred into the next segment's first k-pairs, so
            # the in-order PE and Act queues never drain at seg boundaries.
            state = {"ctxn": None}

            def make_epi(b, qt, h, ctx):
                def epi():
                    if h == 0:
                        state["ctxn"] = ctxnp.tile(
                            [128, QTS], F16, tag="ctxn", name=f"ctxn{b}_{qt}"
                        )
                    ctxn_cur = state["ctxn"]
                    dsb = rcp.tile([1, QTS], F32, tag="dsb")
                    nc.vector.tensor_copy(dsb[:], ctx[DH : DH + 1, :])
                    rcf = rcp.tile([1, QTS], F32, tag="rcf")
                    nc.vector.reciprocal_approx_fast(rcf[:], dsb[:])
                    rcr = rcp.tile([1, QTS], F16, tag="rcr")
                    nc.vector.tensor_copy(rcr[:], rcf[:])
                    bcw = psB.tile([128, QTS], F32, tag="B", name=f"bcw{b}_{qt}_{h}")
                    nc.tensor.matmul(bcw[:], indh_t[:], rcr[:], start=True, stop=True)
                    bc_sb = rcp.tile([64, QTS], F16, tag="bcsb")
                    nc.vector.tensor_copy(bc_sb[:], bcw[0:DH, :])
                    nc.vector.tensor_tensor(
                        ctxn_cur[h * DH : (h + 1) * DH, :], ctx[0:DH, :], bc_sb[:], MULT
                    )
                    if h == 1:
                        for qs in range(QN):
                            ob = outp.tile(
                                [128, D], F16, tag="outb", name=f"ob{b}_{qt}_{qs}"
                            )
                            for oh in range(2):
                                op = psB.tile(
                                    [128, QTS],
                                    F32,
                                    tag="B",
                                    name=f"op{b}_{qt}_{qs}_{oh}",
                                )
                                nc.tensor.matmul(
                                    op[:],
                                    ctxn_cur[:, qs * 128 : (qs + 1) * 128],
                                    wo_t[:, oh * QTS : (oh + 1) * QTS],
                                    start=True,
                                    stop=True,
                                )
                                nc.vector.tensor_copy(
                                    ob[:, oh * QTS : (oh + 1) * QTS], op[:]
                                )
                            r0 = qt * QTS + qs * 128
                            nc.sync.dma_start(out[b, r0 : r0 + 128, :], ob[:])

                return epi

            carry = []  # PV pairs of the previous segment still to emit
            pending_epi = None
            for si, (b, qt, h) in enumerate(segs):
                fetch_eb(si + 1)
                ebt = eb_t.pop((b, qt, h))
                ctx = psB.tile([128, QTS], F32, tag="B", name=f"ctx{b}_{qt}_{h}")
                pending = []
                for kp in range(KPN):
                    sc = psA.tile(
                        [128, 1024], F32, tag="A", name=f"sc{b}_{qt}_{h}_{kp}"
                    )
                    for ki in range(2):
                        kt = 2 * kp + ki
                        nc.tensor.matmul(
                            sc[:, ki * QTS : (ki + 1) * QTS],
                            hk_sb[b][h * DH : (h + 1) * DH, kt * 128 : (kt + 1) * 128],
                            hq_sb[b][h * DH : (h + 1) * DH, qt * QTS : (qt + 1) * QTS],
                            start=True,
                            stop=True,
                        )
                    p0 = p0p.tile([128, 1024], F16, tag="p0")
                    nc.scalar.activation(p0[:], sc[:], EXP)
                    pt = ptp.tile([128, 1024], F16, tag="pt")
                    nc.vector.tensor_tensor(pt[:], p0[:], ebt[:, kp, :], MULT)
                    if carry:
                        carry.pop(0)()
                    elif pending_epi is not None:
                        pending_epi()
                        pending_epi = None
                    pending.append((kp, pt))
                    if kp >= 2:
                        emit_pv(b, h, ctx, *pending.pop(0))
                # defer the last two PV pairs and the epilogue into the next seg
                def mk_pv(bb, hh, cc, item):
                    return lambda: emit_pv(bb, hh, cc, *item)

                carry = [mk_pv(b, h, ctx, item) for item in pending]
                if pending_epi is not None:
                    pending_epi()
                pending_epi = make_epi(b, qt, h, ctx)
                if si < len(chunks):
                    chunks[si]()
            for fn in carry:
                fn()
            pending_epi()

    nc.compile()
    _CACHE["nc"] = nc
    return nc


def _prep_core(core, Wq, Wk, Wv, Wo, shared):
    """Per-core input map. `shared` holds core-independent packed arrays."""
    h0 = core * HPC
    rows = slice(h0 * DH, (h0 + HPC) * DH)

    def packw(w, scale=1.0):
        return np.ascontiguousarray(
            (w[rows].T * scale).reshape(DC, 128, 128).transpose(1, 0, 2)
        ).astype(np.float16)

    # eb[qt, b, hl, p, kp, ki*512+qf] = mask[b, q, k] * exp(pb[h, q, k])
    # with q = qt*512+qf, k = (2*kp+ki)*128+p  (fp16 bit arithmetic in uint16)
    expT, maskT = shared["expT"], shared["maskT"]
    ebc = np.empty((QN, B, HPC, 128, KPN, 1024), np.uint16)
    for qt in range(QN):
        for b in range(B):
            for hl in range(HPC):
                np.multiply(expT[h0 + hl, qt], maskT[b, qt], out=ebc[qt, b, hl])
    return {
        "qT": shared["qT"],
        "kvT": shared["kvT"],
        "indh": shared["indh"],
        "identr": shared["identr"],
        "wq": packw(Wq, 1.0 / np.sqrt(DH)),
        "wk": packw(Wk),
        "wv": packw(Wv),
        "wo": np.ascontiguousarray(Wo[:, rows].T).astype(np.float16),
        "eb": ebc.view(np.float16),
    }


def _prep_shared(query, key_value, mask, position_bias):
    qTp = np.ascontiguousarray(
        query.reshape(B, L, DC, 128).transpose(0, 2, 3, 1)
    ).astype(np.float16)
    kvTp = np.ascontiguousarray(
        key_value.reshape(B, L, DC, 128).transpose(0, 2, 3, 1)
    ).astype(np.float16)
    # [h, q, k] -> [h, qt, p, kp, ki, qf] (fp16 bits as uint16)
    expT = (
        np.ascontiguousarray(
            np.exp(position_bias.astype(np.float32))
            .astype(np.float16)
            .reshape(H, QN, QTS, KPN, 2, 128)
            .transpose(0, 1, 5, 3, 4, 2)
        )
        .view(np.uint16)
        .reshape(H, QN, 128, KPN, 1024)
    )
    maskT = np.ascontiguousarray(
        (np.asarray(mask, dtype=bool))
        .astype(np.uint16)
        .reshape(B, QN, QTS, KPN, 2, 128)
        .transpose(0, 1, 5, 3, 4, 2)
    ).reshape(B, QN, 128, KPN, 1024)
    indh = np.where(np.arange(128) < 64, 1.0, 0.0).astype(np.float16)[None, :]
    return {
        "qT": qTp,
        "kvT": kvTp,
        "expT": expT,
        "maskT": maskT,
        "indh": np.ascontiguousarray(indh),
        "identr": np.eye(128, dtype=np.float32),
    }


def kernel(query, key_value, mask, position_bias, Wq, Wk, Wv, Wo, _trace=False):
    query = np.asarray(query, dtype=np.float32)
    key_value = np.asarray(key_value, dtype=np.float32)
    mask = np.asarray(mask)
    position_bias = np.asarray(position_bias, dtype=np.float32)
    Wq = np.asarray(Wq, dtype=np.float32)
    Wk = np.asarray(Wk, dtype=np.float32)
    Wv = np.asarray(Wv, dtype=np.float32)
    Wo = np.asarray(Wo, dtype=np.float32)

    nc = _build()
    shared = _prep_shared(query, key_value, mask, position_bias)
    in_maps = [_prep_core(c, Wq, Wk, Wv, Wo, shared) for c in range(N_CORES)]
    res = run_bass_kernel_spmd(nc, in_maps, list(range(N_CORES)), trace=_trace)
    _CACHE["last_result"] = res
    acc = res.results[0]["out"].astype(np.float64)
    for c in range(1, N_CORES):
        acc += res.results[c]["out"]
    return acc.astype(np.float32)


# revision 18
# speedup vs baseline: 1.1537x; 1.0902x over previous
"""CPM3 attention kernel for 8 trn2 NeuronCores.

Sharding: tensor-parallel over heads (2 heads/core x both batches).
Device computes per-core partial outputs (Wo row-sharded); host sums.

Structure (v2):
- mask+bias are folded on the host into one multiplicative fp16 table
  eb = mask * exp(position_bias), using exp(s + pb) * mask == exp(s) * eb.
  This removes the per-tile DVE mask/bias fuse and the PSUM identity-matmul
  injection of the additive design (big PE + DVE savings).
- scores are computed transposed [k, q] so the softmax needs no
  partition-dim reductions; denominators come free from an all-ones block
  appended to the transposed-V stationary (one extra PSUM partition).
- per (b, head): the k-sweep runs in 8 chunks of 2 k-tiles: 2 QK matmuls
  into one 2-bank PSUM tile, a single EXP (Act engine) over [128,1024],
  a single fp16 multiply by eb (DVE 2x_1P mode), then 2 PV matmuls.
- hv is transposed k-major via the DMA xbar (SBUF->SBUF), not the PE.
- fp16 operands for all matmuls (2-byte weights use the PE background
  weight-load path and halve HBM traffic); PSUM accumulation stays fp32.
"""

import sys

sys.path.insert(0, "/opt/trn_rl_repo")

import numpy as np

import concourse.bass as bass
import concourse.bacc as bacc
import concourse.tile as tile
import concourse.mybir as mybir
from concourse.bass_utils import run_bass_kernel_spmd

B, L, D, H, DH = 2, 2048, 1024, 16, 64
N_CORES = 8
HPC = H // N_CORES  # heads per core = 2
QTS = 512  # q tile size
QN = L // QTS  # 4
KN = L // 128  # 16 k-tiles
KPN = KN // 2  # 8 k-pairs (2 k-tiles share one 2-bank PSUM score tile)
DC = D // 128  # 8 contraction chunks
HVW = 256  # transposed-V columns per k-tile: [v_h0 | ones64 | v_h1 | ones64]

F32 = mybir.dt.float32
F32R = mybir.dt.float32r
F16 = mybir.dt.float16

_CACHE: dict = {}


def _build():
    if "nc" in _CACHE:
        return _CACHE["nc"]
    nc = bacc.Bacc("TRN2", target_bir_lowering=False, debug=False, num_devices=N_CORES)

    qT = nc.dram_tensor("qT", [B, DC, 128, L], F16, kind="ExternalInput").ap()
    kvT = nc.dram_tensor("kvT", [B, DC, 128, L], F16, kind="ExternalInput").ap()
    wq = nc.dram_tensor("wq", [128, DC, 128], F16, kind="ExternalInput").ap()
    wk = nc.dram_tensor("wk", [128, DC, 128], F16, kind="ExternalInput").ap()
    wv = nc.dram_tensor("wv", [128, DC, 128], F16, kind="ExternalInput").ap()
    wo = nc.dram_tensor("wo", [128, D], F16, kind="ExternalInput").ap()
    eb = nc.dram_tensor(
        "eb", [QN, B, HPC, 128, KPN, 1024], F16, kind="ExternalInput"
    ).ap()
    indh = nc.dram_tensor("indh", [1, 128], F16, kind="ExternalInput").ap()
    identr = nc.dram_tensor("identr", [128, 128], F32R, kind="ExternalInput").ap()
    out = nc.dram_tensor("out", [B, L, D], F16, kind="ExternalOutput").ap()

    EXP = mybir.ActivationFunctionType.Exp
    MULT = mybir.AluOpType.mult

    with tile.TileContext(nc) as tc:
        with (
            tc.tile_pool(name="const", bufs=1) as constp,
            tc.tile_pool(name="stage", bufs=3) as stagep,
            tc.tile_pool(name="hq", bufs=2) as hqp,
            tc.tile_pool(name="hk", bufs=2) as hkp,
            tc.tile_pool(name="hvl", bufs=1) as hvlp,
            tc.tile_pool(name="hvt", bufs=2) as hvtp,
            tc.tile_pool(name="ebp", bufs=3) as ebp,
            tc.tile_pool(name="p0", bufs=8) as p0p,
            tc.tile_pool(name="pt", bufs=8) as ptp,
            tc.tile_pool(name="ctxn", bufs=2) as ctxnp,
            tc.tile_pool(name="rc", bufs=4) as rcp,
            tc.tile_pool(name="outb", bufs=3) as outp,
            tc.tile_pool(name="psA", bufs=2, space=bass.MemorySpace.PSUM) as psA,
            tc.tile_pool(name="psB", bufs=4, space=bass.MemorySpace.PSUM) as psB,
        ):
            # ---- eb prefetch for the first segments (segments are b-major) ----
            segs = [
                (b, qt, h) for b in range(B) for qt in range(QN) for h in range(HPC)
            ]
            eb_t = {}

            def fetch_eb(i):
                if i >= len(segs):
                    return
                b, qt, h = segs[i]
                t = ebp.tile([128, KPN, 1024], F16, tag="eb", name=f"eb{b}_{qt}_{h}")
                nc.gpsimd.dma_start(t[:], eb[qt, b, h])
                eb_t[(b, qt, h)] = t

            # ---- constants (wq first: the first projection needs only it) ----
            wq_t = constp.tile([128, DC, 128], F16, tag="wq")
            nc.sync.dma_start(wq_t[:], wq[:])
            indh_t = constp.tile([1, 128], F16, tag="indh")
            nc.sync.dma_start(indh_t[:], indh[:])
            wk_t = constp.tile([128, DC, 128], F16, tag="wk")
            nc.sync.dma_start(wk_t[:], wk[:])
            wv_t = constp.tile([128, DC, 128], F16, tag="wv")
            nc.sync.dma_start(wv_t[:], wv[:])
            wo_t = constp.tile([128, D], F16, tag="wo")
            nc.sync.dma_start(wo_t[:], wo[:])
            identr_t = constp.tile([128, 128], F32R, tag="identr")
            nc.sync.dma_start(identr_t[:], identr[:])

            def emit_pv(b, h, ctx, kp, pt):
                for ki in range(2):
                    kt = 2 * kp + ki
                    nc.tensor.matmul(
                        ctx[:],
                        hvT[b][:, kt, 2 * h : 2 * h + 2, :],
                        pt[:, ki * QTS : (ki + 1) * QTS],
                        start=(kt == 0),
                        stop=(kt == KN - 1),
                    )

            # ---- prologue helpers (used inline for b0, as interleaved chunks for b1)
            hq_sb, hk_sb, hvT, hvf = {}, {}, {}, {}

            def emit_hq(b, get_c):
                hq_ps = [
                    psA.tile([128, 1024], F32, tag="A", name=f"hqps{b}_{i}")
                    for i in range(2)
                ]
                for dc in range(DC):
                    c = get_c(dc)
                    for qt in range(QN):
                        nc.tensor.matmul(
                            hq_ps[qt // 2][:, (qt % 2) * QTS : (qt % 2 + 1) * QTS],
                            wq_t[:, dc, :],
                            c[:, qt * QTS : (qt + 1) * QTS],
                            start=(dc == 0),
                            stop=(dc == DC - 1),
                        )
                hq_sb[b] = hqp.tile([128, L], F16, tag="hq", name=f"hq{b}")
                for i in range(2):
                    nc.vector.tensor_copy(
                        hq_sb[b][:, i * 1024 : (i + 1) * 1024], hq_ps[i][:]
                    )

            def emit_hk(b, get_c, also_hv):
                hk_ps = [
                    psA.tile([128, 1024], F32, tag="A", name=f"hkps{b}_{i}")
                    for i in range(2)
                ]
                if also_hv:
                    hv_ps = [
                        psB.tile([128, QTS], F32, tag="B", name=f"hvps{b}_{i}")
                        for i in range(QN)
                    ]
                for dc in range(DC):
                    c = get_c(dc)
                    for qt in range(QN):
                        nc.tensor.matmul(
                            hk_ps[qt // 2][:, (qt % 2) * QTS : (qt % 2 + 1) * QTS],
                            wk_t[:, dc, :],
                            c[:, qt * QTS : (qt + 1) * QTS],
                            start=(dc == 0),
                            stop=(dc == DC - 1),
                        )
                        if also_hv:
                            nc.tensor.matmul(
                                hv_ps[qt][:],
                                wv_t[:, dc, :],
                                c[:, qt * QTS : (qt + 1) * QTS],
                                start=(dc == 0),
                                stop=(dc == DC - 1),
                            )
                hk_sb[b] = hkp.tile([128, L], F16, tag="hk", name=f"hk{b}")
                for i in range(2):
                    nc.vector.tensor_copy(
                        hk_sb[b][:, i * 1024 : (i + 1) * 1024], hk_ps[i][:]
                    )
                if also_hv:
                    emit_hv_drain(b, hv_ps)

            def emit_hv(b, get_c):
                hv_ps = [
                    psB.tile([128, QTS], F32, tag="B", name=f"hvps{b}_{i}")
                    for i in range(QN)
                ]
                for dc in range(DC):
                    c = get_c(dc)
                    for qt in range(QN):
                        nc.tensor.matmul(
                            hv_ps[qt][:],
                            wv_t[:, dc, :],
                            c[:, qt * QTS : (qt + 1) * QTS],
                            start=(dc == 0),
                            stop=(dc == DC - 1),
                        )
                emit_hv_drain(b, hv_ps)

            def emit_hv_drain(b, hv_ps):
                hvf[b] = hvlp.tile([128, L], F32R, tag="hvl", name=f"hvf{b}")
                for i in range(QN):
                    nc.vector.tensor_copy(
                        hvf[b][:, i * QTS : (i + 1) * QTS], hv_ps[i][:]
                    )

            def emit_hv_t(b):
                # k-major transposed V, per k-tile [v_h0 | ones64 | v_h1 | ones64]
                # so each head's 128-wide stationary slice carries its values
                # plus denominator (all-ones) columns
                hvT[b] = hvtp.tile([128, KN, 4, 64], F16, tag="hvt", name=f"hvt{b}")
                nc.gpsimd.memset(hvT[b][:].bitcast(mybir.dt.uint16), 0x3C00)
                for kt in range(KN):
                    tp = psB.tile([128, 2, 64], F32R, tag="B", name=f"tp{b}_{kt}")
                    nc.tensor.transpose(
                        tp[:], hvf[b][:, kt * 128 : (kt + 1) * 128], identr_t[:]
                    )
                    nc.vector.tensor_copy(hvT[b][:, kt, 0::2, :], tp[:])

            # ---- prologue: both batches stream q/kv chunks on the sync queue;
            # hk+hv share one kv stream per batch
            def stream_q(b):
                def get(dc):
                    c = stagep.tile([128, L], F16, tag="stage")
                    nc.sync.dma_start(c[:], qT[b, dc])
                    return c

                return get

            def stream_kv(b):
                def get(dc):
                    c = stagep.tile([128, L], F16, tag="stage")
                    nc.sync.dma_start(c[:], kvT[b, dc])
                    return c

                return get

            for b in range(B):
                emit_hq(b, stream_q(b))
                emit_hk(b, stream_kv(b), also_hv=True)
                emit_hv_t(b)

            with tc.tile_wait_until(0.045):
                fetch_eb(0)
            with tc.tile_wait_until(0.055):
                fetch_eb(1)

            # ---- main loop: per (b, q-tile, head) one full k-sweep.
            # PV lags three k-pairs behind QK/exp/mult; the final PV pairs and
            # the normalize/out-project epilogue are split into small parts
            # drained one per k-pair from a global deque, so the in-order PE,
            # Act and DVE queues never see a burst at segment boundaries.
            state = {}

            def make_epi(b, qt, h, ctx):
                if h == 0:
                    state[(b, qt)] = box = {}
                else:
                    box = state[(b, qt)]

                def part_norm():
                    if h == 0:
                        box["ctxn"] = ctxnp.tile(
                            [128, QTS], F16, tag="ctxn", name=f"ctxn{b}_{qt}"
                        )
                    dsb = rcp.tile([1, QTS], F32, tag="dsb")
                    nc.vector.tensor_copy(dsb[:], ctx[DH : DH + 1, :])
                    rcf = rcp.tile([1, QTS], F32, tag="rcf")
                    nc.vector.reciprocal_approx_fast(rcf[:], dsb[:])
                    rcr = rcp.tile([1, QTS], F16, tag="rcr")
                    nc.vector.tensor_copy(rcr[:], rcf[:])
                    bcw = psB.tile([128, QTS], F32, tag="B", name=f"bcw{b}_{qt}_{h}")
                    nc.tensor.matmul(
                        bcw[:], indh_t[:], rcr[:], start=True, stop=True
                    )
                    box["bcw"] = bcw

                def part_ctxn():
                    bc_sb = rcp.tile([64, QTS], F16, tag="bcsb")
                    nc.vector.tensor_copy(bc_sb[:], box["bcw"][0:DH, :])
                    nc.vector.tensor_tensor(
                        box["ctxn"][h * DH : (h + 1) * DH, :],
                        ctx[0:DH, :],
                        bc_sb[:],
                        MULT,
                    )

                parts = [part_norm, part_ctxn]
                if h == 1:

                    def mk_out(qs):
                        def part_out():
                            ctxn_cur = box["ctxn"]
                            ob = outp.tile(
                                [128, D], F16, tag="outb", name=f"ob{b}_{qt}_{qs}"
                            )
                            for oh in range(2):
                                op = psB.tile(
                                    [128, QTS],
                                    F32,
                                    tag="B",
                                    name=f"op{b}_{qt}_{qs}_{oh}",
                                )
                                nc.tensor.matmul(
                                    op[:],
                                    ctxn_cur[:, qs * 128 : (qs + 1) * 128],
                                    wo_t[:, oh * QTS : (oh + 1) * QTS],
                                    start=True,
                                    stop=True,
                                )
                                if qs == 3:
                                    nc.scalar.copy(
                                        ob[:, oh * QTS : (oh + 1) * QTS], op[:]
                                    )
                                else:
                                    nc.vector.tensor_copy(
                                        ob[:, oh * QTS : (oh + 1) * QTS], op[:]
                                    )
                            r0 = qt * QTS + qs * 128
                            nc.sync.dma_start(out[b, r0 : r0 + 128, :], ob[:])

                        return part_out

                    parts += [mk_out(qs) for qs in range(QN)]
                return parts

            carry = []  # PV-pair emitters of the previous segment
            epi_q = []  # deferred epilogue parts (global, ordered)
            LAG = 3
            for si, (b, qt, h) in enumerate(segs):
                fetch_eb(si + 2)
                ebt = eb_t.pop((b, qt, h))
                ctx = psB.tile([128, QTS], F32, tag="B", name=f"ctx{b}_{qt}_{h}")
                pending = []
                for kp in range(KPN):
                    sc = psA.tile(
                        [128, 1024], F32, tag="A", name=f"sc{b}_{qt}_{h}_{kp}"
                    )
                    for ki in range(2):
                        kt = 2 * kp + ki
                        nc.tensor.matmul(
                            sc[:, ki * QTS : (ki + 1) * QTS],
                            hk_sb[b][h * DH : (h + 1) * DH, kt * 128 : (kt + 1) * 128],
                            hq_sb[b][h * DH : (h + 1) * DH, qt * QTS : (qt + 1) * QTS],
                            start=True,
                            stop=True,
                        )
                    p0 = p0p.tile([128, 1024], F16, tag="p0")
                    nc.scalar.activation(p0[:], sc[:], EXP)
                    pt = ptp.tile([128, 1024], F16, tag="pt")
                    nc.vector.tensor_tensor(pt[:], p0[:], ebt[:, kp, :], MULT)
                    if carry:
                        carry.pop(0)()
                    elif epi_q:
                        epi_q.pop(0)()
                    pending.append((kp, pt))
                    if kp >= LAG:
                        emit_pv(b, h, ctx, *pending.pop(0))

                def mk_pv(bb, hh, cc, item):
                    return lambda: emit_pv(bb, hh, cc, *item)

                carry = [mk_pv(b, h, ctx, item) for item in pending]
                epi_q.extend(make_epi(b, qt, h, ctx))
            for fn in carry:
                fn()
            for fn in epi_q:
                fn()

    nc.compile()
    _CACHE["nc"] = nc
    return nc


def _prep_core(core, Wq, Wk, Wv, Wo, shared):
    """Per-core input map. `shared` holds core-independent packed arrays."""
    h0 = core * HPC
    rows = slice(h0 * DH, (h0 + HPC) * DH)

    def packw(w, scale=1.0):
        return np.ascontiguousarray(
            (w[rows].T * scale).reshape(DC, 128, 128).transpose(1, 0, 2)
        ).astype(np.float16)

    # eb[qt, b, hl, p, kp, ki*512+qf] = mask[b, q, k] * exp(pb[h, q, k])
    # with q = qt*512+qf, k = (2*kp+ki)*128+p  (fp16 bit arithmetic in uint16)
    expT, maskT = shared["expT"], shared["maskT"]
    ebc = np.empty((QN, B, HPC, 128, KPN, 1024), np.uint16)
    for qt in range(QN):
        for b in range(B):
            for hl in range(HPC):
                np.multiply(expT[h0 + hl, qt], maskT[b, qt], out=ebc[qt, b, hl])
    return {
        "qT": shared["qT"],
        "kvT": shared["kvT"],
        "indh": shared["indh"],
        "identr": shared["identr"],
        "wq": packw(Wq, 1.0 / np.sqrt(DH)),
        "wk": packw(Wk),
        "wv": packw(Wv),
        "wo": np.ascontiguousarray(Wo[:, rows].T).astype(np.float16),
        "eb": ebc.view(np.float16),
    }


def _prep_shared(query, key_value, mask, position_bias):
    qTp = np.ascontiguousarray(
        query.reshape(B, L, DC, 128).transpose(0, 2, 3, 1)
    ).astype(np.float16)
    kvTp = np.ascontiguousarray(
        key_value.reshape(B, L, DC, 128).transpose(0, 2, 3, 1)
    ).astype(np.float16)
    # [h, q, k] -> [h, qt, p, kp, ki, qf] (fp16 bits as uint16)
    expT = (
        np.ascontiguousarray(
            np.exp(position_bias.astype(np.float32))
            .astype(np.float16)
            .reshape(H, QN, QTS, KPN, 2, 128)
            .transpose(0, 1, 5, 3, 4, 2)
        )
        .view(np.uint16)
        .reshape(H, QN, 128, KPN, 1024)
    )
    maskT = np.ascontiguousarray(
        (np.asarray(mask, dtype=bool))
        .astype(np.uint16)
        .reshape(B, QN, QTS, KPN, 2, 128)
        .transpose(0, 1, 5, 3, 4, 2)
    ).reshape(B, QN, 128, KPN, 1024)
    indh = np.where(np.arange(128) < 64, 1.0, 0.0).astype(np.float16)[None, :]
    return {
        "qT": qTp,
        "kvT": kvTp,
        "expT": expT,
        "maskT": maskT,
        "indh": np.ascontiguousarray(indh),
        "identr": np.eye(128, dtype=np.float32),
    }


def kernel(query, key_value, mask, position_bias, Wq, Wk, Wv, Wo, _trace=False):
    query = np.asarray(query, dtype=np.float32)
    key_value = np.asarray(key_value, dtype=np.float32)
    mask = np.asarray(mask)
    position_bias = np.asarray(position_bias, dtype=np.float32)
    Wq = np.asarray(Wq, dtype=np.float32)
    Wk = np.asarray(Wk, dtype=np.float32)
    Wv = np.asarray(Wv, dtype=np.float32)
    Wo = np.asarray(Wo, dtype=np.float32)

    nc = _build()
    shared = _prep_shared(query, key_value, mask, position_bias)
    in_maps = [_prep_core(c, Wq, Wk, Wv, Wo, shared) for c in range(N_CORES)]
    res = run_bass_kernel_spmd(nc, in_maps, list(range(N_CORES)), trace=_trace)
    _CACHE["last_result"] = res
    acc = res.results[0]["out"].astype(np.float64)
    for c in range(1, N_CORES):
        acc += res.results[c]["out"]
    return acc.astype(np.float32)
